# revision 12
# baseline (speedup 1.0000x reference)
"""Trainium2 Bass kernel v2 for nn_Attention_42288247996512 (sparse causal attention).

Key structural ideas vs baseline:
  1. Mask sparsity: mask[b,s] is shared by queries and keys. A row with m=0
     attends only to itself (eye term) -> out = v[row] EXACTLY. Rows with
     m=1 attend causally to the m=1 rows only. So: permute each batch so
     the ~1024 m=1 rows come first (sorted -> packed causal = plain
     causal), run dense causal attention over KP=1056 packed rows (8 full
     blocks + a 32-row lone block; padding masked), and pass the projected
     V rows through for the m=0 tail.
  2. dm-split sharding: 8 cores = 4 batches x 2 halves of d_model output.
     V projection (the dominant GEMM) is split, not replicated.
  3. bf16 operands everywhere (PE cost model: 1 cyc/row; DMA bytes halved).
     PSUM accum stays f32.
  4. Causal mask tiles generated on-chip (gpsimd affine_select; only 4
     distinct 128-offset patterns exist). Batch padding (packed cols
     [count, KP)) handled by tiny [128,128] additive tiles.
  5. Schedule: R1 = V-proj eighths 0-4 + Q/K projections (K column-major
     so kT[0:512] lands first); R2 = attention pairs software-pipelined,
     V-proj eighths 5-7 (passthrough-only rows) as PE filler to hide
     softmax latency; per-stage outputs combined into single DMAs.
  6. Causally-dead work skipped exactly: even-pair diag chunks are
     256-wide, AV runs per-block 128-wide chains that skip dead key
     blocks, lone block is only LQ=32 query rows.
"""
import math
import threading

import ml_dtypes
import numpy as np

B, S, D, DA = 4, 2048, 1024, 128
P = 128
NCORES = 8
DH = D // 2          # dm half per core
KC = D // P          # 8 contraction chunks
NVB = S // P         # 16 v blocks
BIG = 32768.0        # exactly representable in bf16
KP_DEFAULT = 1056    # packed capacity: 8 full blocks + 32-row lone block
VO0_DEFAULT = 7      # v passthrough blocks [VO0, 16)

_BUILD_LOCK = threading.Lock()
_CACHE: dict = {}


def _build(KP, VO0, PB0):
    from contextlib import ExitStack

    import concourse.mybir as mybir
    import concourse.tile as tile
    from concourse import bacc
    from concourse.masks import make_identity

    dt = mybir.dt
    f32 = dt.float32
    bf16 = dt.bfloat16
    AF = mybir.ActivationFunctionType
    ALU = mybir.AluOpType

    NBK = -(-KP // P)        # packed blocks incl partial lone (9)
    NFB = KP // P            # full blocks (8)
    LQ = KP - NFB * P        # lone-block query width (32), 0 if none
    NPAIR = NFB // 2         # full-block pairs (4)
    LONE = LQ > 0
    PAIR_C = [(p + 2) // 2 for p in range(NPAIR)]   # 512-key chunks per pair
    NLC = -(-KP // 512)      # lone-block chunk count
    LONE_CH = [min(512, KP - 512 * j) for j in range(NLC)] if LONE else []
    NVO = NVB - VO0          # passthrough blocks shipped out
    NPB = NBK - PB0          # pad blocks (may contain masked key columns)

    nc = bacc.Bacc("TRN2", target_bir_lowering=False, debug=False)

    xT = nc.dram_tensor("xT", [D, S], bf16, kind="ExternalInput").ap()
    cT = nc.dram_tensor("cT", [D, KP], bf16, kind="ExternalInput").ap()
    wq = nc.dram_tensor("wq", [P, KC * DA], bf16, kind="ExternalInput").ap()
    wk = nc.dram_tensor("wk", [P, KC * DA], bf16, kind="ExternalInput").ap()
    wvT = nc.dram_tensor("wvT", [D, DH], bf16, kind="ExternalInput").ap()
    pad = nc.dram_tensor("pad", [NPB, P, P], bf16, kind="ExternalInput").ap()

    outT = nc.dram_tensor("outT", [DH, KP], bf16, kind="ExternalOutput").ap()
    den = nc.dram_tensor("den", [P, NBK], f32, kind="ExternalOutput").ap()
    vo = nc.dram_tensor("vo", [P, NVO, DH], bf16, kind="ExternalOutput").ap()

    with tile.TileContext(nc) as tc, ExitStack() as ctx:
        const = ctx.enter_context(tc.tile_pool(name="const", bufs=1))
        persist = ctx.enter_context(tc.tile_pool(name="persist", bufs=1))

        ident_f32 = const.tile([P, P], f32, name="ident_f32")
        make_identity(nc, ident_f32)
        ident = const.tile([P, P], bf16, name="ident")
        nc.vector.tensor_copy(ident[:], ident_f32[:])

        # 4 causal tiles: cz[i][p, j] = 0 if i*128 + p - j >= 0 else -BIG
        cz = []
        for i in range(4):
            t = const.tile([P, 512], bf16, name=f"cz{i}")
            nc.gpsimd.memset(t[:], 0.0)
            nc.gpsimd.affine_select(
                out=t[:], in_=t[:], compare_op=ALU.is_ge, fill=-BIG,
                base=i * P, pattern=[[-1, 512]], channel_multiplier=1)
            cz.append(t)

        wq_sb = const.tile([P, KC, DA], bf16, name="wq_sb")
        wk_sb = const.tile([P, KC, DA], bf16, name="wk_sb")
        wv_sb = const.tile([P, KC, DH], bf16, name="wv_sb")
        pad_sb = const.tile([P, NPB, P], bf16, name="pad_sb")

        kT_sb = persist.tile([P, KP], bf16, name="kT_sb")
        qT_sb = persist.tile([P, KP], bf16, name="qT_sb")
        v_sb = persist.tile([P, NVB, DH], bf16, name="v_sb")
        den_sb = persist.tile([P, NBK], f32, name="den_sb")

        xpool = ctx.enter_context(tc.tile_pool(name="xpool", bufs=3))
        cpool = ctx.enter_context(tc.tile_pool(name="cpool", bufs=8))
        apool = ctx.enter_context(tc.tile_pool(name="apool", bufs=5))
        epool = ctx.enter_context(tc.tile_pool(name="epool", bufs=14))
        dpool = ctx.enter_context(tc.tile_pool(name="dpool", bufs=24))

        xT_r = xT.rearrange("(kc p) s -> p kc s", p=P)
        wvT_r2 = wvT.rearrange("(kc p) m -> p kc m", p=P)

        xtiles = {}

        def load_eighth(se, split=False):
            """DMA one 256-seq-column eighth of permuted x into SBUF."""
            xa = xpool.tile([P, KC, 256], bf16, tag="xt", name=f"xa{se}")
            if split:
                # kc-half xa pieces interleaved with dm-half wv pieces:
                # the first matmuls need only wv cols [0:256] (v halves).
                for g in range(4):
                    # bigger wv piece first: the matmul gates on the later
                    # of (xa, wv), so the smaller xa transfer goes last
                    nc.scalar.dma_start(wv_sb[:, 2 * g:2 * g + 2, :],
                                        wvT_r2[:, 2 * g:2 * g + 2, :])
                    nc.sync.dma_start(
                        xa[:, 2 * g:2 * g + 2, :],
                        xT_r[:, 2 * g:2 * g + 2, se * 256:(se + 1) * 256])
            elif split is None:
                for g in range(2):
                    nc.sync.dma_start(
                        xa[:, 4 * g:4 * g + 4, :],
                        xT_r[:, 4 * g:4 * g + 4,
                             se * 256:(se + 1) * 256])
            else:
                nc.sync.dma_start(
                    xa[:, :, :], xT_r[:, :, se * 256:(se + 1) * 256])
            xtiles[se] = xa

        def v_eighth_mm(se, pool, tagpfx="psv", halves=False):
            """V projection matmuls for eighth se (2 seq blocks x DH).
            halves=True: dm-halves sequentially, so only half of wv gates
            the start (startup DMA/PE balance)."""
            xa = xtiles[se]
            pss = [pool.tile([P, DH], f32, tag=f"{tagpfx}{sb}",
                             name=f"{tagpfx}{se}_{sb}") for sb in range(2)]
            if halves:
                for h in range(2):
                    c0 = h * 256
                    for kc in range(KC):
                        for sb in range(2):
                            nc.tensor.matmul(
                                pss[sb][:, c0:c0 + 256],
                                lhsT=xa[:, kc, sb * P:(sb + 1) * P],
                                rhs=wv_sb[:, kc, c0:c0 + 256],
                                start=(kc == 0), stop=(kc == KC - 1),
                            )
            else:
                for kc in range(KC):
                    for sb in range(2):
                        nc.tensor.matmul(
                            pss[sb][:],
                            lhsT=xa[:, kc, sb * P:(sb + 1) * P],
                            rhs=wv_sb[:, kc, :],
                            start=(kc == 0), stop=(kc == KC - 1),
                        )
            for sb in range(2):
                bi = se * 2 + sb
                nc.any.tensor_copy(v_sb[:, bi, :], pss[sb][:])

        def v_eighth_mm_sliced(se, pool):
            """Per-kc slices of v_eighth_mm for use as PE filler."""
            xa = xtiles[se]
            pss = [pool.tile([P, DH], f32, tag=f"fil{sb}",
                             name=f"fil{se}_{sb}") for sb in range(2)]

            def slice_kc(kc):
                for sb in range(2):
                    nc.tensor.matmul(
                        pss[sb][:],
                        lhsT=xa[:, kc, sb * P:(sb + 1) * P],
                        rhs=wv_sb[:, kc, :],
                        start=(kc == 0), stop=(kc == KC - 1),
                    )
                if kc == KC - 1:
                    for sb in range(2):
                        bi = se * 2 + sb
                        nc.any.tensor_copy(v_sb[:, bi, :], pss[sb][:])
                        nc.sync.dma_start(vo[:, bi - VO0, :], v_sb[:, bi, :])
            return [lambda kc=kc: slice_kc(kc) for kc in range(KC)]

        # ---------------- R1: projections ----------------
        with ExitStack() as phase1:
            psv_pool = phase1.enter_context(
                tc.tile_pool(name="psv", bufs=2, space="PSUM"))
            pj_pool = phase1.enter_context(
                tc.tile_pool(name="pj", bufs=1, space="PSUM"))

            def q_chunk(tag, xparts, q0):
                """Project q cols [q0, q0+sum(width)) from x eighth tiles."""
                w = sum(wd for _, _, wd in xparts)
                ps = pj_pool.tile([P, 512], f32, tag=tag,
                                  name=f"psq_{tag}_{q0}")
                col = 0
                for (se, off, wd) in xparts:
                    for kc in range(KC):
                        nc.tensor.matmul(
                            ps[:, col:col + wd],
                            lhsT=wq_sb[:, kc, :],
                            rhs=xtiles[se][:, kc, off:off + wd],
                            start=(kc == 0), stop=(kc == KC - 1),
                        )
                    col += wd
                nc.any.tensor_copy(qT_sb[:, q0:q0 + w], ps[:, :w])

            ctiles = {}

            def load_ct(kc):
                ctile = cpool.tile([P, KP], bf16, tag="ct", name=f"ct{kc}")
                nc.sync.dma_start(ctile[:], cT[kc * P:(kc + 1) * P, :])
                ctiles[kc] = ctile

            def k_pass(ps, psoff, col, wd, copy_engine):
                for kc in range(KC):
                    nc.tensor.matmul(
                        ps[:, psoff:psoff + wd],
                        lhsT=wk_sb[:, kc, :],
                        rhs=ctiles[kc][:, col:col + wd],
                        start=(kc == 0), stop=(kc == KC - 1),
                    )
                copy_engine.tensor_copy(kT_sb[:, col:col + wd],
                                        ps[:, psoff:psoff + wd])

            load_eighth(0, split=True)
            load_eighth(1, split=None)
            nc.sync.dma_start(
                wq_sb[:], wq.rearrange("p (kc m) -> p kc m", kc=KC))
            nc.sync.dma_start(pad_sb[:], pad.rearrange("s p t -> p s t"))
            v_eighth_mm(0, psv_pool)
            load_eighth(2, split=None)
            v_eighth_mm(1, psv_pool)
            nc.sync.dma_start(
                wk_sb[:], wk.rearrange("p (kc m) -> p kc m", kc=KC))
            q_chunk("pjA", [(0, 0, 256), (1, 0, 256)], 0)
            load_eighth(3, split=None)
            for kc in range(4):
                load_ct(kc)
            v_eighth_mm(2, psv_pool)
            load_eighth(4, split=None)
            for kc in range(4, KC):
                load_ct(kc)
            v_eighth_mm(3, psv_pool)
            q_chunk("pjB", [(2, 0, 256), (3, 0, 256)], 512)
            v_eighth_mm(4, psv_pool)
            if LQ:
                q_chunk("pjC", [(4, 0, LQ)], 1024)

            # K projection, column-major: kT[0:256] (all pair 0 needs) is
            # copied out first so attention starts while K passes still run.
            psA = pj_pool.tile([P, 512], f32, tag="pjA", name="psk0")
            k_pass(psA, 0, 0, 256, nc.vector)
            k_pass(psA, 256, 256, 256, nc.vector)
            k_pass(pj_pool.tile([P, 512], f32, tag="pjB", name="psk1"),
                   0, 512, 512, nc.any)
            if LQ:
                k_pass(pj_pool.tile([P, 512], f32, tag="pjC",
                                    name="psk2"), 0, 1024, LQ, nc.any)

            # late x eighths (passthrough rows) + vo for R1-computed blocks
            load_eighth(5)
            load_eighth(6)
            load_eighth(7)
            for bi in range(VO0, 10):
                nc.sync.dma_start(vo[:, bi - VO0, :], v_sb[:, bi, :])

        # ---------------- R2: attention ----------------
        eTs_all = {}   # pr -> list of (eT tile, off, width) in kb order
        dacs_all = {}  # block idx -> list of dac tiles

        def apply_masks(psl, k0, wd, qbase, name, rows=P):
            """Additive causal+padding mask; returns exp-source AP list
            [(src_ap, col0, width)] covering [0, wd)."""
            diag = k0 <= qbase < k0 + wd  # causal boundary inside chunk
            pads = []
            for bi in range(max(PB0, k0 // P), -(-(k0 + wd) // P)):
                if bi * P >= KP:
                    break
                pads.append(bi)
            if diag:
                base = qbase - k0
                czt = cz[base // P]
                sbl = apool.tile([P, 512], f32, tag="sbl", name=f"sbl{name}")
                nc.vector.tensor_tensor(out=sbl[:rows, :wd],
                                        in0=psl[:rows, :wd],
                                        in1=czt[:rows, :wd], op=ALU.add)
                for bi in pads:
                    c0 = bi * P - k0
                    pw = min(P, wd - c0)
                    nc.vector.tensor_tensor(
                        out=sbl[:rows, c0:c0 + pw],
                        in0=sbl[:rows, c0:c0 + pw],
                        in1=pad_sb[:rows, bi - PB0, :pw], op=ALU.add)
                return [(sbl, 0, wd)]
            if not pads:
                return [(psl, 0, wd)]
            # no causal add: exp psl directly outside pad cols, add inside
            segs = []
            cur = 0
            for bi in pads:
                c0 = bi * P - k0
                pw = min(P, wd - c0)
                if c0 > cur:
                    segs.append((psl, cur, c0 - cur))
                sbl = apool.tile([P, P], f32, tag="sblp",
                                 name=f"sblp{name}_{bi}")
                nc.vector.tensor_tensor(out=sbl[:rows, :pw],
                                        in0=psl[:rows, c0:c0 + pw],
                                        in1=pad_sb[:rows, bi - PB0, :pw],
                                        op=ALU.add)
                segs.append((sbl, c0, pw))
                cur = c0 + pw
            if cur < wd:
                segs.append((psl, cur, wd - cur))
            return segs

        with tc.tile_pool(name="psl", bufs=2, space="PSUM") as psl_pool, \
             tc.tile_pool(name="psT", bufs=2, space="PSUM") as psT_pool, \
             tc.tile_pool(name="psav", bufs=2, space="PSUM") as psav_pool, \
             tc.tile_pool(name="fill", bufs=1, space="PSUM") as fill_pool:

            # PE filler queue: eighths 5-7 sliced per-kc (24 slices)
            filler = []
            for se in (5, 6, 7):
                filler.extend(v_eighth_mm_sliced(se, fill_pool))
            filler_pos = [0]

            def take_filler(n):
                i = filler_pos[0]
                for f in filler[i:i + n]:
                    f()
                filler_pos[0] = min(i + n, len(filler))

            def do_exp(segs, e, slot, name, rows=P):
                """exp each masked segment into e; accumulate dacs."""
                for i, (src, c0, w) in enumerate(segs):
                    dac = dpool.tile([P, 1], f32, tag="dac",
                                     name=f"dac{name}_{i}")
                    soff = 0 if src.shape[-1] == w else c0
                    nc.scalar.activation(
                        e[:rows, c0:c0 + w], src[:rows, soff:soff + w],
                        AF.Exp, scale=1.0, accum_out=dac[:rows, :])
                    dacs_all.setdefault(slot, []).append(dac)

            def stage_a_pair(pr, j, mid_hook=None):
                """Pair pr (blocks 2pr,2pr+1), key chunk j. The diag chunk
                of an even pair only needs 256 key cols (the rest is fully
                causally masked): skip logits/exp/transpose/AV for them."""
                c = PAIR_C[pr]
                W = 512
                if j == c - 1 and pr % 2 == 0:
                    W = 256
                nk = W // P
                es = []
                for blk in range(2):
                    slot = pr * 2 + blk
                    psl = psl_pool.tile([P, 512], f32, tag="psl",
                                        name=f"psl{slot}_{j}")
                    nc.tensor.matmul(
                        psl[:, :W],
                        lhsT=qT_sb[:, slot * P:(slot + 1) * P],
                        rhs=kT_sb[:, j * 512:j * 512 + W],
                        start=True, stop=True,
                    )
                    segs = apply_masks(psl, j * 512, W, slot * P,
                                       f"{slot}_{j}")
                    e = apool.tile([P, 512], bf16, tag="e",
                                   name=f"e{slot}_{j}")
                    do_exp(segs, e, slot, f"{slot}_{j}")
                    es.append(e)
                if mid_hook is not None:
                    mid_hook()
                is_diag = (j == c - 1)
                psTs = [psT_pool.tile([P, 512], bf16, tag="psT",
                                      name=f"psT{pr}_{j}_{h}")
                        for h in range(-(-nk // 2))]
                for blk in range(2):
                    for ks in range(nk):
                        if blk == 0 and is_diag and ks == nk - 1:
                            continue  # fully causally masked for block 0
                        nc.tensor.transpose(
                            psTs[ks // 2][:, (ks % 2) * 256 + blk * P:
                                          (ks % 2) * 256 + (blk + 1) * P],
                            es[blk][:, ks * P:(ks + 1) * P],
                            ident[:],
                        )
                for h in range(-(-nk // 2)):
                    eT = epool.tile([P, 512], bf16, tag="eT",
                                    name=f"eT{pr}_{j}_{h}")
                    hk = min(2, nk - h * 2)  # ks-subblocks in this psT
                    nc.any.tensor_copy(eT[:, :hk * 256], psTs[h][:, :hk * 256])
                    lst = eTs_all.setdefault(pr, [])
                    for s in range(hk):
                        ks = h * 2 + s
                        live0 = not (is_diag and ks == nk - 1)
                        lst.append((eT, s * 256, live0))

            def stage_a_lone(j, mid_hook=None):
                """Lone partial block (LQ rows), chunk j (widths LONE_CH)."""
                slot = NBK - 1
                wd = LONE_CH[j]
                psl = psl_pool.tile([P, 512], f32, tag="psl",
                                    name=f"pslL_{j}")
                nc.tensor.matmul(
                    psl[:LQ, :wd],
                    lhsT=qT_sb[:, NFB * P:NFB * P + LQ],
                    rhs=kT_sb[:, j * 512:j * 512 + wd],
                    start=True, stop=True,
                )
                segs = apply_masks(psl, j * 512, wd, NFB * P, f"L{j}",
                                   rows=LQ)
                nk = -(-wd // P)
                e = apool.tile([P, 512], bf16, tag="e", name=f"eL_{j}")
                do_exp(segs, e, slot, f"L{j}", rows=LQ)
                if mid_hook is not None:
                    mid_hook()
                psT = psT_pool.tile([P, 512], bf16, tag="psT",
                                    name=f"psTL_{j}")
                for ks in range(nk):
                    kw = min(P, wd - ks * P)
                    nc.tensor.transpose(
                        psT[:kw, ks * LQ:ks * LQ + LQ],
                        e[:LQ, ks * P:ks * P + kw],
                        ident[:LQ, :LQ],
                    )
                eT = epool.tile([P, 512], bf16, tag="eT", name=f"eTL_{j}")
                nc.any.tensor_copy(eT[:, :nk * LQ], psT[:, :nk * LQ])
                lst = eTs_all.setdefault("L", [])
                for ks in range(nk):
                    lst.append((eT, ks * LQ, True))

            def finish_den(slots, dma=False):
                for slot in slots:
                    dl = dacs_all[slot]
                    dst = den_sb[:, slot:slot + 1]
                    if len(dl) == 1:
                        nc.any.tensor_copy(dst, dl[0][:])
                    else:
                        nc.vector.tensor_tensor(out=dst, in0=dl[0][:],
                                                in1=dl[1][:], op=ALU.add)
                        for d in dl[2:]:
                            nc.vector.tensor_tensor(out=dst, in0=dst,
                                                    in1=d[:], op=ALU.add)
                if dma:
                    nc.sync.dma_start(den[:], den_sb[:])

            outT_r = outT.rearrange("(dmc p) q -> p dmc q", p=P)

            def stage_b(pr, qw=256, dmcs=(0, 1, 2, 3), pool=None,
                        ptags=("psav",), copy_eng=None):
                """AV for pair pr (or 'L'): out.T[dm, q] = v.T @ e.T.
                Per-block 128-wide chains; blk0 skips its dead kbs."""
                eTs = eTs_all[pr]
                q0 = (pr * 256) if pr != "L" else NFB * P
                pool = pool or psav_pool
                if pr == "L":
                    chains = [(0, qw, 0, list(range(len(eTs))))]
                else:
                    live0 = [k for k, (_, _, lv) in enumerate(eTs) if lv]
                    chains = [(0, P, 0, live0),
                              (P, P, P, list(range(len(eTs))))]
                osb = apool.tile([P, len(dmcs), 256], bf16, tag="osb",
                                 name=f"osb{pr}_{dmcs[0]}")
                for i, dmc in enumerate(dmcs):
                    psav = pool.tile([P, 256], f32,
                                     tag=ptags[dmc % len(ptags)],
                                     name=f"psav{pr}_{dmc}")
                    for (c0, cw, roff, kbs) in chains:
                        for n, kb in enumerate(kbs):
                            eT, off, _ = eTs[kb]
                            kr = min(P, KP - kb * P)
                            nc.tensor.matmul(
                                psav[:, c0:c0 + cw],
                                lhsT=v_sb[:kr, kb, dmc * P:(dmc + 1) * P],
                                rhs=eT[:kr, off + roff:off + roff + cw],
                                start=(n == 0), stop=(n == len(kbs) - 1),
                            )
                    (copy_eng or nc.any).tensor_copy(osb[:, i, :qw],
                                                     psav[:, :qw])
                nc.sync.dma_start(
                    outT_r[:, dmcs[0]:dmcs[0] + len(dmcs), q0:q0 + qw],
                    osb[:, :, :qw])

            # pipeline: cover each stage_a's softmax latency with PE work;
            # keep a 256-wide stage_b last so the final DMA chain overlaps
            stage_a_pair(0, 0, mid_hook=lambda: take_filler(5))
            stage_a_pair(1, 0, mid_hook=lambda: take_filler(5))
            stage_a_pair(2, 0, mid_hook=lambda: (stage_b(0),
                                                 finish_den([0, 1])))
            stage_a_pair(2, 1, mid_hook=lambda: take_filler(5))
            stage_a_pair(3, 0, mid_hook=lambda: (stage_b(1),
                                                 finish_den([2, 3])))
            stage_a_pair(3, 1, mid_hook=lambda: take_filler(3))
            if LONE:
                stage_a_lone(0, mid_hook=lambda: (
                    stage_b(2, dmcs=(0, 1)), finish_den([4, 5])))
                stage_a_lone(1, mid_hook=lambda: take_filler(2))
                stage_a_lone(2, mid_hook=lambda: (
                    stage_b(2, dmcs=(2, 3)), take_filler(2),
                    finish_den([6, 7])))
                stage_b(3, dmcs=(0, 1))
                take_filler(2)
                finish_den([NBK - 1], dma=True)
                stage_b(3, dmcs=(2, 3))
                stage_b("L", qw=LQ, pool=fill_pool, ptags=("fil0", "fil1"),
                        copy_eng=nc.vector)
            else:
                stage_b(2)
                finish_den([4, 5])
                stage_b(3)
                finish_den([6, 7], dma=True)
            take_filler(24)

    nc.compile()
    return nc


def _get_nc(KP=KP_DEFAULT, VO0=VO0_DEFAULT, PB0=VO0_DEFAULT):
    with _BUILD_LOCK:
        key = (KP, VO0, PB0)
        if key not in _CACHE:
            _CACHE[key] = _build(KP, VO0, PB0)
        return _CACHE[key]


def kernel(x, cross, Wq, Wk, Wv, mask):
    from concourse import bass_utils

    x = np.asarray(x, dtype=np.float32)
    cross = np.asarray(cross, dtype=np.float32)
    mask = np.asarray(mask)
    scale = 1.0 / math.sqrt(DA)

    counts = mask.sum(axis=1).astype(int)
    cmax = int(counts.max())
    if cmax <= 1024:
        KP = 1024
    else:
        KP = 1024 + -(-(cmax - 1024) // 32) * 32
        if KP > 1024 + P:
            raise NotImplementedError(f"packed count {cmax} exceeds 1152")
    KP = max(KP, KP_DEFAULT)
    VO0 = min(VO0_DEFAULT, int(counts.min()) // P)
    PB0 = int(counts.min()) // P
    nc = _get_nc(KP, VO0, PB0)
    NBK = -(-KP // P)
    NVO = NVB - VO0
    NPB = NBK - PB0

    bf = ml_dtypes.bfloat16
    # weights shared across cores; pre-arrange so DMA rows are contiguous
    wqT = (np.asarray(Wq, np.float32) * scale).T  # [D, DA]
    wkT = np.asarray(Wk, np.float32).T
    wq_h = np.ascontiguousarray(
        wqT.reshape(KC, P, DA).transpose(1, 0, 2).reshape(P, KC * DA)
    ).astype(bf)
    wk_h = np.ascontiguousarray(
        wkT.reshape(KC, P, DA).transpose(1, 0, 2).reshape(P, KC * DA)
    ).astype(bf)
    wvT_full = np.ascontiguousarray(np.asarray(Wv, np.float32).T)  # [D, D]

    perms = []
    batch_data = []
    for b in range(B):
        m = mask[b].astype(bool)
        perm = np.argsort(~m, kind="stable")  # m=1 rows first, ascending
        perms.append(perm)
        xp = x[b][perm]                        # [S, D]
        xT_h = np.ascontiguousarray(xp.T).astype(bf)
        cp = cross[b][perm[:KP]]               # [KP, D]
        cT_h = np.ascontiguousarray(cp.T).astype(bf)
        # pad tiles: additive -BIG on packed key cols >= count
        kneg = np.zeros(-(-KP // P) * P, np.float32)
        kneg[counts[b]:] = -BIG
        pad_h = np.broadcast_to(
            kneg[PB0 * P:].reshape(NPB, 1, P), (NPB, P, P))
        batch_data.append((xT_h, cT_h,
                           np.ascontiguousarray(pad_h).astype(bf)))

    in_maps = []
    for core in range(NCORES):
        b, half = divmod(core, 2)
        xT_h, cT_h, pad_h = batch_data[b]
        in_maps.append({
            "xT": xT_h,
            "cT": cT_h,
            "wq": wq_h,
            "wk": wk_h,
            "wvT": np.ascontiguousarray(
                wvT_full[:, half * DH:(half + 1) * DH]).astype(bf),
            "pad": pad_h,
        })

    res = bass_utils.run_bass_kernel_spmd(
        nc, in_maps, core_ids=list(range(NCORES)))

    out = np.empty((B, S, D), np.float32)
    for core in range(NCORES):
        b, half = divmod(core, 2)
        r = res.results[core]
        cnt = counts[b]
        perm = perms[b]
        att = r["outT"].astype(np.float32).T          # [KP, DH]
        denf = r["den"].astype(np.float32).T.reshape(-1)  # [NBK*P]
        sl = slice(half * DH, (half + 1) * DH)
        out[b, perm[:cnt], sl] = att[:cnt] / denf[:cnt, None]
        vrows = r["vo"].astype(np.float32).transpose(1, 0, 2).reshape(
            NVO * P, DH)                               # rows VO0*128..2048
        out[b, perm[cnt:], sl] = vrows[cnt - VO0 * P:]
    return out
